# revision 1
# baseline (speedup 1.0000x reference)
"""Trainium2 Bass kernel for pre-LN multi-head GQA attention (B=2, S=2048, H=2048,
NH=16, D=128, NKV=4, causal, RoPE).

Sharding: 8 cores = 2 batches x 4 KV groups. Core c handles batch c//4 and KV head
c%4 (its 4 query heads; Wq/Wk/Wv column-sharded by head, Wo row-sharded). Each core
computes a partial output [S, H]; the host sums the 4 per-batch partials.

Device dataflow (per core, per 512-wide s-chunk):
  One pass over x^T tiles feeds BOTH the LayerNorm statistics (ones-matmuls into
  PSUM -> per-column mean/rstd, rows born broadcast) AND the raw projections
  Qraw^T/Kraw^T/Vraw^T = W^T x^T.  LayerNorm is then applied by linearity:
      proj(y) = a ⊙_s proj(x) + wsum ⊗ b,   a = rstd[s], b = -mean[s]*rstd[s]
  via one fused DVE fixup per projection tile (wsum precomputed on host).
  RoPE on Q^T/K^T (partition-half ops, host sin/cos tables); V^T -> V by PE
  transpose.  Attention per head in the transposed layout: logits^T = K^T.T Q^T,
  exp on ACT (1/sqrt(D) folded into activation scale), causal mask via
  affine_select, denominator via ones-matmul (broadcast rows), ctx^T = V.T expP,
  normalize by DVE reciprocal.  Output projection ctx^T.T @ Wo -> [S, H] partial.
All matmuls are float32r (full PE rate at N=512); PSUM accumulation is fp32.
"""

import sys

for p in ("/opt/trn_rl_repo",):
    if p not in sys.path:
        sys.path.append(p)

import numpy as np

import concourse.bass as bass
import concourse.tile as tile
from concourse import bacc
from concourse import mybir
from concourse.masks import make_identity

F32 = mybir.dt.float32
F32R = mybir.dt.float32r
ALU = mybir.AluOpType
ACTF = mybir.ActivationFunctionType

B, S, H = 2, 2048, 2048
NH, D, NKV = 16, 128, 4
G = NH // NKV  # query heads per KV head (= heads per core)
EPS = 1e-6
MIN_WIN, MAX_WIN = 1.0, 10000.0
SCALE = 1.0 / float(np.sqrt(np.float32(D)))
CHUNK = 512  # s-chunk width
NCH = S // CHUNK  # 4 chunks
HC = H // 128  # 16 h-chunks




def build_program(has_bias: bool) -> bass.Bass:
    nc = bacc.Bacc(
        "TRN2",
        target_bir_lowering=False,
        debug=False,
        enable_asserts=False,
        num_devices=8,
    )
    xT = nc.dram_tensor("xT", [H, S], F32R, kind="ExternalInput").ap()
    wq = nc.dram_tensor("wq", [H, G * D], F32R, kind="ExternalInput").ap()
    wk = nc.dram_tensor("wk", [H, D], F32R, kind="ExternalInput").ap()
    wv = nc.dram_tensor("wv", [H, D], F32R, kind="ExternalInput").ap()
    wo = nc.dram_tensor("wo", [G * D, H], F32R, kind="ExternalInput").ap()
    cos_d = nc.dram_tensor("cos_t", [128, S], F32, kind="ExternalInput").ap()
    sin_d = nc.dram_tensor("sin_t", [128, S], F32, kind="ExternalInput").ap()
    wqs = nc.dram_tensor("wqsum", [G * D], F32, kind="ExternalInput").ap()
    wks = nc.dram_tensor("wksum", [D], F32, kind="ExternalInput").ap()
    wvs = nc.dram_tensor("wvsum", [D], F32, kind="ExternalInput").ap()
    prot = nc.dram_tensor("prot", [128, 128], F32R, kind="ExternalInput").ap()
    onesd = nc.dram_tensor("onesc", [128, 128], F32R, kind="ExternalInput").ap()
    bq = nc.dram_tensor("bq", [G * D], F32, kind="ExternalInput").ap()
    bk = nc.dram_tensor("bk", [D], F32, kind="ExternalInput").ap()
    bv = nc.dram_tensor("bv", [D], F32, kind="ExternalInput").ap()
    outp = nc.dram_tensor("outp", [S, H], F32, kind="ExternalOutput").ap()

    with tile.TileContext(nc) as tc:
        with (
            tc.tile_pool(name="singles", bufs=1) as singles,
            tc.tile_pool(name="xp", bufs=2) as xp,
            tc.tile_pool(name="work", bufs=4) as work,
            tc.tile_pool(name="evp", bufs=7) as evp,
            tc.tile_pool(name="tabs", bufs=2) as tabs,
            tc.tile_pool(name="qp", bufs=5) as qp,
            tc.tile_pool(name="cp", bufs=5) as cp,
            tc.tile_pool(name="ep", bufs=3) as ep,
            tc.tile_pool(name="abp", bufs=4) as abp,
            tc.tile_pool(name="stg", bufs=2) as stg,
            tc.tile_pool(name="psum", bufs=4, space="PSUM") as psum,
        ):
            # ---- resident weights / constants ----
            # Small constants go FIRST on the sync ring (ahead of x tiles) so
            # the first stats matmul isn't gated by the weight FIFO; weights
            # load on the scalar ring ordered by first use (Wo last, Wq split
            # per head so each head's projections start as its slice lands).
            ones_sb = singles.tile([128, 128], F32R)
            nc.sync.dma_start(ones_sb, onesd)
            prot_sb = singles.tile([128, 128], F32R)
            nc.sync.dma_start(prot_sb, prot)
            wk_sb = singles.tile([128, HC, D], F32R)
            nc.gpsimd.dma_start(wk_sb, wk.rearrange("(hc p) q -> p hc q", p=128))
            wv_sb = singles.tile([128, HC, D], F32R)
            nc.gpsimd.dma_start(wv_sb, wv.rearrange("(hc p) q -> p hc q", p=128))
            wqs_sb = singles.tile([128, G], F32)
            nc.gpsimd.dma_start(wqs_sb, wqs.rearrange("(g p) -> p g", p=128))
            wks_sb = singles.tile([128, 1], F32)
            nc.gpsimd.dma_start(wks_sb, wks[:, None])
            wvs_sb = singles.tile([128, 1], F32)
            nc.gpsimd.dma_start(wvs_sb, wvs[:, None])
            wq_sb = singles.tile([128, HC, G, D], F32R)
            for g_ in range(G):
                nc.gpsimd.dma_start(
                    wq_sb[:, :, g_, :],
                    wq[:, g_ * D : (g_ + 1) * D].rearrange(
                        "(hc p) q -> p hc q", p=128
                    ),
                )
            wo_sb = singles.tile([128, G, H], F32R)
            nc.gpsimd.dma_start(wo_sb, wo.rearrange("(g p) h -> p g h", p=128))
            kT_sb = singles.tile([128, S], F32R)  # roped K^T, filled per chunk
            v_sb = singles.tile([128, S // 128, D], F32R)  # V natural, per k-tile
            ident = singles.tile([128, 128], F32)
            make_identity(nc, ident)
            eps_sb = singles.tile([128, 1], F32)
            nc.gpsimd.memset(eps_sb, EPS)
            if has_bias:
                bq_sb = singles.tile([128, G], F32)
                nc.gpsimd.dma_start(bq_sb, bq.rearrange("(g p) -> p g", p=128))
                bk_sb = singles.tile([128, 1], F32)
                nc.gpsimd.dma_start(bk_sb, bk[:, None])
                bv_sb = singles.tile([128, 1], F32)
                nc.gpsimd.dma_start(bv_sb, bv[:, None])

            def rope(out, raw, cos_c, sin_c, rps, bank):
                """out = RoPE(raw) for one [128, CHUNK] head tile.
                rot = P_rot @ raw on PE gives [-t2; t1]; then
                out = raw*cos + rot*sin with all operands partition-aligned.
                rps is a shared per-chunk PSUM tile (alternating banks)."""
                nc.tensor.matmul(rps[:, bank, :], prot_sb, raw, start=True, stop=True)
                tmp = work.tile([128, CHUNK], F32, tag="ropetmp", name="rt1")
                nc.vector.tensor_mul(tmp, rps[:, bank, :], sin_c)
                nc.gpsimd.tensor_mul(out, raw, cos_c)
                nc.vector.tensor_add(out, out, tmp)

            for qb in range(NCH):
                sl = slice(qb * CHUNK, (qb + 1) * CHUNK)
                cos_c = tabs.tile([128, CHUNK], F32, tag="cos")
                nc.gpsimd.dma_start(cos_c, cos_d[:, sl])
                sin_c = tabs.tile([128, CHUNK], F32, tag="sin")
                nc.gpsimd.dma_start(sin_c, sin_d[:, sl])

                # ---- A+B fused: LN stats and raw projections in one x pass ----
                st = psum.tile([128, 2, CHUNK], F32, tag="ps")
                pq01 = psum.tile([128, 2, CHUNK], F32, tag="ps")
                pq23 = psum.tile([128, 2, CHUNK], F32, tag="ps")
                pkv = psum.tile([128, 2, CHUNK], F32, tag="ps")
                for hx in range(4):
                    xt4 = xp.tile([128, 4, CHUNK], F32R, tag="x")
                    nc.sync.dma_start(
                        xt4,
                        xT[hx * 512 : (hx + 1) * 512, sl].rearrange(
                            "(j p) s -> p j s", p=128
                        ),
                    )
                    for j in range(4):
                        hc = hx * 4 + j
                        xt = xt4[:, j, :]
                        xr = xt
                        s0, s1 = hc == 0, hc == HC - 1
                        nc.tensor.matmul(st[:, 0, :], ones_sb, xr, start=s0, stop=s1)
                        sq = work.tile([128, CHUNK], F32R, tag="sq", name="sq")
                        nc.scalar.square(sq, xt)
                        nc.tensor.matmul(st[:, 1, :], ones_sb, sq, start=s0, stop=s1)
                        nc.tensor.matmul(pq01[:, 0, :], wq_sb[:, hc, 0, :], xr, start=s0, stop=s1)
                        nc.tensor.matmul(pq01[:, 1, :], wq_sb[:, hc, 1, :], xr, start=s0, stop=s1)
                        nc.tensor.matmul(pq23[:, 0, :], wq_sb[:, hc, 2, :], xr, start=s0, stop=s1)
                        nc.tensor.matmul(pq23[:, 1, :], wq_sb[:, hc, 3, :], xr, start=s0, stop=s1)
                        nc.tensor.matmul(pkv[:, 0, :], wk_sb[:, hc, :], xr, start=s0, stop=s1)
                        nc.tensor.matmul(pkv[:, 1, :], wv_sb[:, hc, :], xr, start=s0, stop=s1)

                # stats -> a = rstd (bcast rows), b = -mean*rstd (bcast rows)
                mean = work.tile([128, CHUNK], F32, tag="stat", name="mean")
                nc.vector.tensor_scalar_mul(mean, st[:, 0, :], 1.0 / H)
                msq = work.tile([128, CHUNK], F32, tag="stat", name="msq")
                nc.vector.tensor_mul(msq, mean, mean)
                var = work.tile([128, CHUNK], F32, tag="stat", name="var")
                nc.vector.scalar_tensor_tensor(
                    out=var, in0=st[:, 1, :], scalar=1.0 / H, in1=msq,
                    op0=ALU.mult, op1=ALU.subtract,
                )
                std = work.tile([128, CHUNK], F32, tag="stat", name="std")
                nc.scalar.activation(std, var, ACTF.Sqrt, bias=eps_sb)
                a_t = abp.tile([128, CHUNK], F32, tag="ab")
                nc.vector.reciprocal(a_t, std)
                b_t = abp.tile([128, CHUNK], F32, tag="ab")
                nc.vector.scalar_tensor_tensor(
                    out=b_t, in0=mean, scalar=-1.0, in1=a_t,
                    op0=ALU.mult, op1=ALU.mult,
                )

                def fixup(dst, wsum_col, bias_col):
                    # in-place: dst = a ⊙ dst + wsum_col ⊗ b  (+ bias_col)
                    nc.vector.tensor_mul(dst, dst, a_t)
                    nc.vector.scalar_tensor_tensor(
                        out=dst,
                        in0=b_t,
                        scalar=wsum_col,
                        in1=dst,
                        op0=ALU.mult,
                        op1=ALU.add,
                    )
                    if bias_col is not None:
                        nc.vector.tensor_scalar_add(dst, dst, bias_col)

                # evict all projection PSUMs first (no stats dependency) so the
                # next chunk's fused pass gets PSUM slots back quickly; K's
                # fixup+rope chain goes first because attention needs it first.
                kraw = evp.tile([128, CHUNK], F32R, tag="ev", name="kraw")
                nc.scalar.copy(kraw, pkv[:, 0, :])
                vt = evp.tile([128, CHUNK], F32, tag="ev", name="vt")
                nc.scalar.copy(vt, pkv[:, 1, :])
                qraws = []
                for head in range(G):
                    ps_v = (pq01 if head < 2 else pq23)[:, head % 2, :]
                    raw = evp.tile([128, CHUNK], F32R, tag="ev", name="qraw")
                    nc.scalar.copy(raw, ps_v)
                    qraws.append(raw)
                fixup(kraw, wks_sb[:, 0:1], bk_sb[:, 0:1] if has_bias else None)
                rps = psum.tile([128, 2, CHUNK], F32, tag="ps", name="rps")
                rope(kT_sb[:, sl], kraw, cos_c, sin_c, rps, 0)
                qts = []
                for head in range(G):
                    raw = qraws[head]
                    fixup(raw, wqs_sb[:, head : head + 1],
                          bq_sb[:, head : head + 1] if has_bias else None)
                    q = qp.tile([128, CHUNK], F32R, tag="q")
                    rope(q, raw, cos_c, sin_c, rps, (head + 1) % 2)
                    qts.append(q)
                fixup(vt, wvs_sb[:, 0:1], bv_sb[:, 0:1] if has_bias else None)
                pt = psum.tile([128, 2, CHUNK], F32, tag="ps")
                for j in range(4):
                    nc.tensor.transpose(
                        pt[:, j // 2, (j % 2) * 128 : (j % 2 + 1) * 128],
                        vt[:, j * 128 : (j + 1) * 128],
                        ident,
                    )
                nc.scalar.copy(v_sb[:, qb * 4 : qb * 4 + 4, :], pt[:, :, 0:256])

                # ---- C: causal attention for this q-chunk ----
                kmax = 4 * (qb + 1)
                ctxs = []
                for head in range(G):
                    pcd = psum.tile([128, 2, CHUNK], F32, tag="ps")
                    for kb2 in range(0, kmax, 2):
                        pl = psum.tile([128, 2, CHUNK], F32, tag="ps")
                        for i in (0, 1):
                            kb = kb2 + i
                            nc.tensor.matmul(
                                pl[:, i, :],
                                kT_sb[:, kb * 128 : (kb + 1) * 128],
                                qts[head],
                                start=True,
                                stop=True,
                            )
                        e = ep.tile([128, 2, CHUNK], F32R, tag="e")
                        nc.scalar.activation(e, pl, ACTF.Exp, scale=SCALE)
                        for i in (0, 1):
                            kb = kb2 + i
                            if kb >= 4 * qb:
                                # keep where q_idx - k_idx >= 0: j - p - off >= 0
                                off = kb * 128 - qb * CHUNK
                                nc.gpsimd.affine_select(
                                    out=e[:, i, :],
                                    in_=e[:, i, :],
                                    compare_op=ALU.is_ge,
                                    fill=0.0,
                                    base=-off,
                                    pattern=[[1, CHUNK]],
                                    channel_multiplier=-1,
                                )
                            s0, s1 = kb == 0, kb == kmax - 1
                            nc.tensor.matmul(
                                pcd[:, 1, :], ones_sb, e[:, i, :], start=s0, stop=s1
                            )
                            nc.tensor.matmul(
                                pcd[:, 0, :], v_sb[:, kb, :], e[:, i, :], start=s0, stop=s1
                            )
                    rec = cp.tile([128, CHUNK], F32, tag="ctx", name="rec")
                    nc.vector.reciprocal(rec, pcd[:, 1, :])
                    ctx = cp.tile([128, CHUNK], F32R, tag="ctx", name="ctx")
                    nc.vector.tensor_mul(ctx, pcd[:, 0, :], rec)
                    ctxs.append(ctx)

                # ---- D: output projection for this s-chunk ----
                for sm in range(4):
                    row = slice(qb * CHUNK + sm * 128, qb * CHUNK + (sm + 1) * 128)
                    for nc2 in range(0, 4, 2):
                        stage = stg.tile([128, 2 * CHUNK], F32, tag="o")
                        po = psum.tile([128, 2, CHUNK], F32, tag="ps")
                        for half in (0, 1):
                            col = slice((nc2 + half) * 512, (nc2 + half + 1) * 512)
                            for cc in range(G):
                                nc.tensor.matmul(
                                    po[:, half, :],
                                    ctxs[cc][:, sm * 128 : (sm + 1) * 128],
                                    wo_sb[:, cc, col],
                                    start=cc == 0,
                                    stop=cc == G - 1,
                                )
                            nc.scalar.copy(
                                stage[:, half * CHUNK : (half + 1) * CHUNK],
                                po[:, half, :],
                            )
                        nc.sync.dma_start(
                            outp[row, nc2 * 512 : (nc2 + 2) * 512], stage
                        )
    nc.compile()
    return nc


_PROGRAMS: dict[bool, bass.Bass] = {}


def get_program(has_bias: bool) -> bass.Bass:
    if has_bias not in _PROGRAMS:
        _PROGRAMS[has_bias] = build_program(has_bias)
    return _PROGRAMS[has_bias]


def make_in_maps(x, ln_gamma, ln_beta, Wq, Wk, Wv, Wo):
    x = np.asarray(x, np.float32)
    g = np.asarray(ln_gamma, np.float32)
    be = np.asarray(ln_beta, np.float32)
    Wq = np.asarray(Wq, np.float32)
    Wk = np.asarray(Wk, np.float32)
    Wv = np.asarray(Wv, np.float32)
    Wo = np.asarray(Wo, np.float32)

    Wqg = Wq * g[:, None]
    Wkg = Wk * g[:, None]
    Wvg = Wv * g[:, None]
    bq_full = be @ Wq
    bk_full = be @ Wk
    bv_full = be @ Wv
    wqsum = Wqg.sum(axis=0)
    wksum = Wkg.sum(axis=0)
    wvsum = Wvg.sum(axis=0)
    has_bias = bool(np.any(be != 0.0))

    half = D // 2
    ts = MIN_WIN * (MAX_WIN / MIN_WIN) ** (
        2.0 * np.arange(half, dtype=np.float32) / D
    )
    ang = np.arange(S, dtype=np.float32)[None, :] / ts[:, None].astype(np.float32)
    cos_t = np.cos(ang).astype(np.float32)
    sin_t = np.sin(ang).astype(np.float32)
    cos_t = np.concatenate([cos_t, cos_t], axis=0)  # [128, S]
    sin_t = np.concatenate([sin_t, sin_t], axis=0)

    prot = np.zeros((128, 128), np.float32)
    for m in range(64):
        prot[m + 64, m] = -1.0
        prot[m, m + 64] = 1.0

    xT = [np.ascontiguousarray(x[b].T) for b in range(B)]
    in_maps = []
    for c in range(8):
        b, h = divmod(c, NKV)
        qs = slice(h * G * D, (h + 1) * G * D)
        ks = slice(h * D, (h + 1) * D)
        in_maps.append(
            {
                "xT": xT[b],
                "wq": np.ascontiguousarray(Wqg[:, qs]),
                "wk": np.ascontiguousarray(Wkg[:, ks]),
                "wv": np.ascontiguousarray(Wvg[:, ks]),
                "wo": np.ascontiguousarray(Wo[qs, :]),
                "prot": prot,
                "onesc": np.ones((128, 128), np.float32),
                "cos_t": cos_t,
                "sin_t": sin_t,
                "wqsum": np.ascontiguousarray(wqsum[qs]),
                "wksum": np.ascontiguousarray(wksum[ks]),
                "wvsum": np.ascontiguousarray(wvsum[ks]),
                "bq": np.ascontiguousarray(bq_full[qs]),
                "bk": np.ascontiguousarray(bk_full[ks]),
                "bv": np.ascontiguousarray(bv_full[ks]),
            }
        )
    return in_maps, has_bias


def kernel(x, ln_gamma, ln_beta, Wq, Wk, Wv, Wo):
    from concourse.bass_utils import run_bass_kernel_spmd

    in_maps, has_bias = make_in_maps(x, ln_gamma, ln_beta, Wq, Wk, Wv, Wo)
    nc = get_program(has_bias)
    res = run_bass_kernel_spmd(nc, in_maps, core_ids=list(range(8)))
    outs = [m["outp"] for m in res.results]
    out = np.empty((B, S, H), np.float32)
    for b in range(B):
        out[b] = (outs[NKV * b] + outs[NKV * b + 1]) + (
            outs[NKV * b + 2] + outs[NKV * b + 3]
        )
    return out



# revision 5
# speedup vs baseline: 1.4682x; 1.4682x over previous
"""Trainium2 Bass kernel for pre-LN multi-head GQA attention (B=2, S=2048, H=2048,
NH=16, D=128, NKV=4, causal, RoPE).

Sharding: 8 cores = 2 batches x 4 KV groups. Core c handles batch c//4 and KV head
c%4 (its 4 query heads; Wq/Wk/Wv column-sharded by head, Wo row-sharded). Each core
computes a partial output [S, H]; the host sums the 4 per-batch partials.

v2: software-pipelined emission. Per 512-wide s-chunk qb the projection x-pass
(stats + K/V in pass A, Q heads in pass B, x tiles cached in SBUF between passes)
is woven instruction-by-instruction with the previous chunk's attention so the PE
never drains on the exp/select dependency chains. All output projections (Wo) are
deferred to a drain phase where they fill the last chunk's attention stalls.
Activations/weights are bf16 (PE rate is identical to f32r, but DMA and SBUF
halve); PSUM accumulation stays fp32. Diagonal attention tiles are width-restricted
to >=256 columns (causal saving at full fp32r/bf16 PE rate). LayerNorm is applied
by linearity: proj(y) = a * proj(x) + wsum x b, rstd computed as exp(-0.5*ln(var+eps))
so the ACT engine only ever uses one activation table (no table swaps).
"""

import sys

for p in ("/opt/trn_rl_repo",):
    if p not in sys.path:
        sys.path.append(p)

import numpy as np

import concourse.bass as bass
import concourse.tile as tile
from concourse import bacc
from concourse import mybir
from concourse.masks import make_identity

F32 = mybir.dt.float32
BF16 = mybir.dt.bfloat16
ALU = mybir.AluOpType
ACTF = mybir.ActivationFunctionType

B, S, H = 2, 2048, 2048
NH, D, NKV = 16, 128, 4
G = NH // NKV  # query heads per KV head (= heads per core)
EPS = 1e-6
MIN_WIN, MAX_WIN = 1.0, 10000.0
SCALE = 1.0 / float(np.sqrt(np.float32(D)))
CHUNK = 512
NCH = S // CHUNK  # 4
HC = H // 128  # 16

MM_NS = 213.0  # 512-wide full-rate matmul


def build_program(has_bias: bool) -> bass.Bass:
    nc = bacc.Bacc(
        "TRN2",
        target_bir_lowering=False,
        debug=False,
        enable_asserts=False,
        num_devices=8,
    )
    xT = nc.dram_tensor("xT", [H, S], BF16, kind="ExternalInput").ap()
    # host pre-arranged: [128, HC*G*D], [128, HC*D], [128, G*H]
    wq = nc.dram_tensor("wq", [128, HC * G * D], BF16, kind="ExternalInput").ap()
    wk = nc.dram_tensor("wk", [128, HC * D], BF16, kind="ExternalInput").ap()
    wv = nc.dram_tensor("wv", [128, HC * D], BF16, kind="ExternalInput").ap()
    wo = nc.dram_tensor("wo", [128, G * H], BF16, kind="ExternalInput").ap()
    cos_d = nc.dram_tensor("cos_t", [128, S], F32, kind="ExternalInput").ap()
    sin_d = nc.dram_tensor("sin_t", [128, S], F32, kind="ExternalInput").ap()
    wqs = nc.dram_tensor("wqsum", [128, G], F32, kind="ExternalInput").ap()
    wks = nc.dram_tensor("wksum", [128, 1], F32, kind="ExternalInput").ap()
    wvs = nc.dram_tensor("wvsum", [128, 1], F32, kind="ExternalInput").ap()
    bq = nc.dram_tensor("bq", [128, G], F32, kind="ExternalInput").ap()
    bk = nc.dram_tensor("bk", [128, 1], F32, kind="ExternalInput").ap()
    bv = nc.dram_tensor("bv", [128, 1], F32, kind="ExternalInput").ap()
    outp = nc.dram_tensor("outp", [S, H], F32, kind="ExternalOutput").ap()

    with tile.TileContext(nc) as tc:
        with (
            tc.tile_pool(name="singles", bufs=1) as singles,
            tc.tile_pool(name="xp", bufs=5) as xp,
            tc.tile_pool(name="wrk", bufs=4) as wrk,
            tc.tile_pool(name="sqp", bufs=2) as sqp,
            tc.tile_pool(name="abp", bufs=4) as abp,
            tc.tile_pool(name="tabs", bufs=4) as tabs,
            tc.tile_pool(name="qp", bufs=9) as qp,
            tc.tile_pool(name="evp", bufs=6) as evp,
            tc.tile_pool(name="ep", bufs=4) as ep,
            tc.tile_pool(name="cp", bufs=17) as cp,
            tc.tile_pool(name="rcp", bufs=2) as rcp,
            tc.tile_pool(name="stg", bufs=3) as stg,
            tc.tile_pool(name="psum", bufs=4, space="PSUM") as psum,
        ):
            # ---- on-chip constants (no DMA) ----
            ones_sb = singles.tile([128, 128], BF16)
            nc.gpsimd.memset(ones_sb, 1.0)
            ident = singles.tile([128, 128], BF16)
            make_identity(nc, ident)
            prot_sb = singles.tile([128, 128], BF16)
            nc.gpsimd.memset(prot_sb, 0.0)
            # +1 at (m, m+64): keep where col - row - 64 != 0 else fill 1
            nc.gpsimd.affine_select(
                out=prot_sb, in_=prot_sb, compare_op=ALU.not_equal, fill=1.0,
                base=-64, pattern=[[1, 128]], channel_multiplier=-1,
            )
            # -1 at (m+64, m): keep where col - row + 64 != 0 else fill -1
            nc.gpsimd.affine_select(
                out=prot_sb, in_=prot_sb, compare_op=ALU.not_equal, fill=-1.0,
                base=64, pattern=[[1, 128]], channel_multiplier=-1,
            )
            eps_sb = singles.tile([128, 1], F32)
            nc.gpsimd.memset(eps_sb, EPS)

            # ---- resident weights ----
            wk_sb = singles.tile([128, HC, D], BF16)
            nc.gpsimd.dma_start(wk_sb, wk.rearrange("p (hc q) -> p hc q", hc=HC))
            wv_sb = singles.tile([128, HC, D], BF16)
            nc.gpsimd.dma_start(wv_sb, wv.rearrange("p (hc q) -> p hc q", hc=HC))
            wqs_sb = singles.tile([128, G], F32)
            nc.gpsimd.dma_start(wqs_sb, wqs)
            wks_sb = singles.tile([128, 1], F32)
            nc.gpsimd.dma_start(wks_sb, wks)
            wvs_sb = singles.tile([128, 1], F32)
            nc.gpsimd.dma_start(wvs_sb, wvs)
            if has_bias:
                bq_sb = singles.tile([128, G], F32)
                nc.gpsimd.dma_start(bq_sb, bq)
                bk_sb = singles.tile([128, 1], F32)
                nc.gpsimd.dma_start(bk_sb, bk)
                bv_sb = singles.tile([128, 1], F32)
                nc.gpsimd.dma_start(bv_sb, bv)
            wo_sb = singles.tile([128, G, H], BF16)
            nc.gpsimd.dma_start(wo_sb, wo.rearrange("p (g h) -> p g h", g=G))
            kT_sb = singles.tile([128, S], BF16)  # roped K^T, filled per chunk
            v_sb = singles.tile([128, S // 128, D], BF16)  # V natural, per k-tile
            # wq on the sync ring, after chunk-0 x tiles (emitted in F(0)).
            wq_sb = singles.tile([128, HC, G, D], BF16)

            # cross-stream state
            qts_all: dict[int, list] = {}  # chunk -> [4 roped Q tiles]
            ctx_all: dict[int, list] = {}  # chunk -> [4 ctx^T bf16 tiles]
            ab_t: dict[int, tuple] = {}  # chunk -> (a_t, b_t)

            def rope(out, raw, cos_c, sin_c):
                """out = RoPE(raw) for one [128, CHUNK] bf16 tile."""
                rps = psum.tile([128, CHUNK], F32, tag="pl", name="rps", bufs=2)
                nc.tensor.matmul(rps, prot_sb, raw, start=True, stop=True)
                tmp = wrk.tile([128, CHUNK], F32, tag="wrk", name="ropetmp")
                nc.vector.tensor_mul(tmp, rps, sin_c)
                nc.gpsimd.tensor_mul(out, raw, cos_c)
                nc.vector.tensor_add(out, out, tmp)

            def fixup(dst, src_ps, a_t, b_t, wsum_col, bias_col):
                # dst = a * src + wsum_col x b  (+ bias_col); evicts PSUM
                nc.vector.tensor_mul(dst, src_ps, a_t)
                nc.vector.scalar_tensor_tensor(
                    out=dst, in0=b_t, scalar=wsum_col, in1=dst,
                    op0=ALU.mult, op1=ALU.add,
                )
                if bias_col is not None:
                    nc.vector.tensor_scalar_add(dst, dst, bias_col)

            # ---------------- x-pass (filler stream F) ----------------
            def xpass_units(qb):
                sl = slice(qb * CHUNK, (qb + 1) * CHUNK)
                st: dict = {}

                def u_dma():
                    st["xt"] = []
                    for hx in range(4):
                        xt4 = xp.tile([128, 4, CHUNK], BF16, tag="x", name="xt4")
                        nc.sync.dma_start(
                            xt4,
                            xT[hx * 512 : (hx + 1) * 512, sl].rearrange(
                                "(j p) s -> p j s", p=128
                            ),
                        )
                        st["xt"].append(xt4)
                    if qb == 0:
                        nc.sync.dma_start(
                            wq_sb,
                            wq.rearrange("p (hc g q) -> p hc g q", hc=HC, g=G),
                        )
                    cos_c = tabs.tile([128, CHUNK], F32, tag="cos")
                    nc.gpsimd.dma_start(cos_c, cos_d[:, sl])
                    sin_c = tabs.tile([128, CHUNK], F32, tag="sin")
                    nc.gpsimd.dma_start(sin_c, sin_d[:, sl])
                    st["cos"], st["sin"] = cos_c, sin_c
                    st["sum"] = psum.tile([128, CHUNK], F32, tag="xacc", name="psum_s")
                    st["ssq"] = psum.tile([128, CHUNK], F32, tag="xacc", name="psum_q")
                    st["k"] = psum.tile([128, CHUNK], F32, tag="xacc", name="psum_k")
                    st["v"] = psum.tile([128, CHUNK], F32, tag="xacc", name="psum_v")

                units = [(1.0, u_dma)]

                # pass A: stats + K + V
                def mk_a(hc):
                    def u():
                        xt = st["xt"][hc // 4][:, hc % 4, :]
                        s0, s1 = hc == 0, hc == HC - 1
                        sq = sqp.tile([128, CHUNK], BF16, tag="sq", name="sq")
                        nc.scalar.square(sq, xt)
                        nc.tensor.matmul(st["sum"], ones_sb, xt, start=s0, stop=s1)
                        nc.tensor.matmul(st["ssq"], ones_sb, sq, start=s0, stop=s1)
                        nc.tensor.matmul(st["k"], wk_sb[:, hc, :], xt, start=s0, stop=s1)
                        nc.tensor.matmul(st["v"], wv_sb[:, hc, :], xt, start=s0, stop=s1)
                    return u

                units += [(4 * MM_NS, mk_a(hc)) for hc in range(HC)]

                def u_stats():
                    mean = wrk.tile([128, CHUNK], F32, tag="wrk", name="mean")
                    nc.vector.tensor_scalar_mul(mean, st["sum"], 1.0 / H)
                    msq = wrk.tile([128, CHUNK], F32, tag="wrk", name="msq")
                    nc.vector.tensor_mul(msq, mean, mean)
                    var = wrk.tile([128, CHUNK], F32, tag="wrk", name="var")
                    nc.vector.scalar_tensor_tensor(
                        out=var, in0=st["ssq"], scalar=1.0 / H, in1=msq,
                        op0=ALU.mult, op1=ALU.subtract,
                    )
                    lnv = wrk.tile([128, CHUNK], F32, tag="wrk", name="lnv")
                    nc.scalar.activation(lnv, var, ACTF.Ln, bias=eps_sb)
                    a_t = abp.tile([128, CHUNK], F32, tag="ab", name="a_t")
                    nc.scalar.activation(a_t, lnv, ACTF.Exp, scale=-0.5)
                    b_t = abp.tile([128, CHUNK], F32, tag="ab", name="b_t")
                    nc.vector.scalar_tensor_tensor(
                        out=b_t, in0=mean, scalar=-1.0, in1=a_t,
                        op0=ALU.mult, op1=ALU.mult,
                    )
                    ab_t[qb] = (a_t, b_t)
                    # K/V fixups (DVE only; PE stream continues with pass B)
                    kraw = evp.tile([128, CHUNK], BF16, tag="ev", name="kraw")
                    fixup(kraw, st["k"], a_t, b_t, wks_sb[:, 0:1],
                          bk_sb[:, 0:1] if has_bias else None)
                    vt = evp.tile([128, CHUNK], BF16, tag="ev", name="vt")
                    fixup(vt, st["v"], a_t, b_t, wvs_sb[:, 0:1],
                          bv_sb[:, 0:1] if has_bias else None)
                    st["kraw"], st["vt"] = kraw, vt

                units.append((1.0, u_stats))

                # pass B: Q heads (reuses cached x tiles)
                def u_allocq():
                    st["q"] = [
                        psum.tile([128, CHUNK], F32, tag="xacc", name=f"psum_q{g_}")
                        for g_ in range(G)
                    ]

                units.append((1.0, u_allocq))

                def mk_b(hc):
                    def u():
                        xt = st["xt"][hc // 4][:, hc % 4, :]
                        s0, s1 = hc == 0, hc == HC - 1
                        for g_ in range(G):
                            nc.tensor.matmul(
                                st["q"][g_], wq_sb[:, hc, g_, :], xt,
                                start=s0, stop=s1,
                            )
                    return u

                units += [(4 * MM_NS, mk_b(hc)) for hc in range(HC)]

                def u_kv_pe():
                    # K rope into resident kT, V transpose into resident v_sb
                    rope(kT_sb[:, sl], st["kraw"], st["cos"], st["sin"])
                    pt = psum.tile([128, 4, 128], BF16, tag="pl", name="pt", bufs=2)
                    for j in range(4):
                        nc.tensor.transpose(
                            pt[:, j, :], st["vt"][:, j * 128 : (j + 1) * 128], ident
                        )
                    nc.gpsimd.tensor_copy(v_sb[:, qb * 4 : qb * 4 + 4, :], pt)

                units.append((2 * MM_NS, u_kv_pe))

                def mk_q(g_):
                    def u():
                        a_t, b_t = ab_t[qb]
                        raw = evp.tile([128, CHUNK], BF16, tag="ev", name="qraw")
                        fixup(raw, st["q"][g_], a_t, b_t,
                              wqs_sb[:, g_ : g_ + 1],
                              bq_sb[:, g_ : g_ + 1] if has_bias else None)
                        q = qp.tile([128, CHUNK], BF16, tag="q")
                        rope(q, raw, st["cos"], st["sin"])
                        qts_all.setdefault(qb, []).append(q)
                    return u

                units += [(MM_NS, mk_q(g_)) for g_ in range(G)]
                return units

            # ---------------- attention (dependent stream D) ----------------
            def attn_units(a):
                kmax = 4 * (a + 1)
                units = []
                for h in range(G):
                    hs: dict = {}

                    def mk_start(h=h, hs=hs):
                        def u():
                            hs["den"] = psum.tile(
                                [128, CHUNK], F32, tag="cd", name="den", bufs=2
                            )
                            hs["ctx"] = psum.tile(
                                [128, CHUNK], F32, tag="cd", name="ctxp", bufs=2
                            )
                            hs["pl"] = {}
                            hs["e"] = {}
                        return u

                    def owidth(kb):
                        kbloc = kb - 4 * a
                        if kbloc < 0:
                            return 0, CHUNK
                        off = min(kbloc * 128, 256)
                        return off, CHUNK - off

                    def mk_l(kb, h=h, hs=hs):
                        off, w = owidth(kb)

                        def u():
                            pl = psum.tile([128, CHUNK], F32, tag="pl", name="pl", bufs=2)
                            hs["pl"][kb] = pl
                            nc.tensor.matmul(
                                pl[:, off : off + w],
                                kT_sb[:, kb * 128 : (kb + 1) * 128],
                                qts_all[a][h][:, off : off + w],
                                start=True, stop=True,
                            )
                        return u

                    def mk_edc(kb, h=h, hs=hs):
                        off, w = owidth(kb)
                        kbloc = kb - 4 * a
                        s0, s1 = kb == 0, kb == kmax - 1

                        def u():
                            pl = hs["pl"].pop(kb)
                            e = ep.tile([128, CHUNK], BF16, tag="e", name="e")
                            nc.scalar.activation(
                                e[:, off : off + w], pl[:, off : off + w],
                                ACTF.Exp, scale=SCALE,
                            )
                            if kbloc >= 0:
                                nc.gpsimd.affine_select(
                                    out=e[:, off : off + w],
                                    in_=e[:, off : off + w],
                                    compare_op=ALU.is_ge,
                                    fill=0.0,
                                    base=off - kbloc * 128,
                                    pattern=[[1, w]],
                                    channel_multiplier=-1,
                                )
                            nc.tensor.matmul(
                                hs["den"][:, off : off + w], ones_sb,
                                e[:, off : off + w], start=s0, stop=s1,
                            )
                            nc.tensor.matmul(
                                hs["ctx"][:, off : off + w], v_sb[:, kb, :],
                                e[:, off : off + w], start=s0, stop=s1,
                            )
                        return u

                    units.append((1.0, mk_start()))
                    # pipeline: L0 L1 EDC0 L2 EDC1 ... L(kmax-1) EDC(kmax-2) EDC(kmax-1)
                    lws = [owidth(kb)[1] for kb in range(kmax)]
                    units.append((MM_NS * lws[0] / 512, mk_l(0)))
                    if kmax > 1:
                        units.append((MM_NS * lws[1] / 512, mk_l(1)))
                    for kb in range(kmax):
                        units.append((2 * MM_NS * lws[kb] / 512, mk_edc(kb)))
                        if kb + 2 < kmax:
                            units.append(
                                (MM_NS * lws[kb + 2] / 512, mk_l(kb + 2))
                            )

                    def mk_norm(h=h, hs=hs):
                        def u():
                            rec = rcp.tile([128, CHUNK], F32, tag="rec", name="rec")
                            nc.vector.reciprocal(rec, hs["den"])
                            ctx = cp.tile([128, CHUNK], BF16, tag="ctx", name="ctx")
                            nc.vector.tensor_mul(ctx, hs["ctx"], rec)
                            ctx_all.setdefault(a, []).append(ctx)
                        return u

                    units.append((1.0, mk_norm()))
                return units

            # ---------------- output projection (drain filler) ----------------
            def wo_units(c):
                units = []
                for sm in range(4):
                    for pair in range(2):
                        ps: dict = {}

                        def mk_half(half, sm=sm, pair=pair, ps=ps):
                            nc2 = 2 * pair + half
                            col = slice(nc2 * 512, (nc2 + 1) * 512)
                            row = slice(c * CHUNK + sm * 128,
                                        c * CHUNK + (sm + 1) * 128)

                            def u():
                                if half == 0:
                                    ps["stage"] = stg.tile(
                                        [128, 2 * CHUNK], F32, tag="o", name="stage"
                                    )
                                po = psum.tile([128, CHUNK], F32, tag="xacc",
                                               name="po")
                                for cc in range(G):
                                    nc.tensor.matmul(
                                        po,
                                        ctx_all[c][cc][:, sm * 128 : (sm + 1) * 128],
                                        wo_sb[:, cc, col],
                                        start=cc == 0, stop=cc == G - 1,
                                    )
                                nc.gpsimd.tensor_copy(
                                    ps["stage"][:, half * CHUNK : (half + 1) * CHUNK],
                                    po,
                                )
                                if half == 1:
                                    nc.sync.dma_start(
                                        outp[row, pair * 1024 : (pair + 1) * 1024],
                                        ps["stage"],
                                    )
                            return u

                        units.append((4 * MM_NS, mk_half(0)))
                        units.append((4 * MM_NS, mk_half(1)))
                return units

            # ---------------- proportional weave ----------------
            def weave(dep, fill):
                td = sum(u[0] for u in dep) or 1.0
                tf = sum(u[0] for u in fill) or 1.0
                i = j = 0
                ad = af = 0.0
                while i < len(dep) or j < len(fill):
                    if i < len(dep) and (j >= len(fill) or ad / td <= af / tf):
                        ad += dep[i][0]
                        dep[i][1]()
                        i += 1
                    else:
                        af += fill[j][0]
                        fill[j][1]()
                        j += 1

            # ---------------- schedule ----------------
            for u in xpass_units(0):
                u[1]()
            for qb in range(1, NCH):
                weave(attn_units(qb - 1), xpass_units(qb))
            drain_fill = []
            for c in range(NCH - 1):
                drain_fill += wo_units(c)
            weave(attn_units(NCH - 1), drain_fill)
            for u in wo_units(NCH - 1):
                u[1]()
    nc.compile()
    return nc


_PROGRAMS: dict[bool, bass.Bass] = {}


def get_program(has_bias: bool) -> bass.Bass:
    if has_bias not in _PROGRAMS:
        _PROGRAMS[has_bias] = build_program(has_bias)
    return _PROGRAMS[has_bias]


def make_in_maps(x, ln_gamma, ln_beta, Wq, Wk, Wv, Wo):
    import ml_dtypes

    BF = ml_dtypes.bfloat16
    x = np.asarray(x, np.float32)
    g = np.asarray(ln_gamma, np.float32)
    be = np.asarray(ln_beta, np.float32)
    Wq = np.asarray(Wq, np.float32)
    Wk = np.asarray(Wk, np.float32)
    Wv = np.asarray(Wv, np.float32)
    Wo = np.asarray(Wo, np.float32)

    Wqg = (Wq * g[:, None]).astype(BF)
    Wkg = (Wk * g[:, None]).astype(BF)
    Wvg = (Wv * g[:, None]).astype(BF)
    Wo_b = Wo.astype(BF)
    bq_full = be @ Wq
    bk_full = be @ Wk
    bv_full = be @ Wv
    # column sums of the bf16-rounded weights (device computes with those)
    wqsum = Wqg.astype(np.float32).sum(axis=0)
    wksum = Wkg.astype(np.float32).sum(axis=0)
    wvsum = Wvg.astype(np.float32).sum(axis=0)
    has_bias = bool(np.any(be != 0.0))

    half = D // 2
    ts = MIN_WIN * (MAX_WIN / MIN_WIN) ** (
        2.0 * np.arange(half, dtype=np.float32) / D
    )
    ang = np.arange(S, dtype=np.float32)[None, :] / ts[:, None].astype(np.float32)
    cos_t = np.cos(ang).astype(np.float32)
    sin_t = np.sin(ang).astype(np.float32)
    cos_t = np.ascontiguousarray(np.concatenate([cos_t, cos_t], axis=0))  # [128, S]
    sin_t = np.ascontiguousarray(np.concatenate([sin_t, sin_t], axis=0))

    xT = [np.ascontiguousarray(x[b].T).astype(BF) for b in range(B)]

    def arrange_w(w, ncol):
        # [H, ncol] -> [128, HC*ncol] matching sbuf [128, HC, ncol]
        return np.ascontiguousarray(
            w.reshape(HC, 128, ncol).transpose(1, 0, 2).reshape(128, HC * ncol)
        )

    in_maps = []
    for c in range(8):
        b, h = divmod(c, NKV)
        qs = slice(h * G * D, (h + 1) * G * D)
        ks = slice(h * D, (h + 1) * D)
        wo_slice = Wo_b[qs, :]  # [G*D, H]
        wo_arr = np.ascontiguousarray(
            wo_slice.reshape(G, 128, H).transpose(1, 0, 2).reshape(128, G * H)
        )
        in_maps.append(
            {
                "xT": xT[b],
                "wq": arrange_w(Wqg[:, qs], G * D),
                "wk": arrange_w(Wkg[:, ks], D),
                "wv": arrange_w(Wvg[:, ks], D),
                "wo": wo_arr,
                "cos_t": cos_t,
                "sin_t": sin_t,
                "wqsum": np.ascontiguousarray(
                    wqsum[qs].reshape(G, 128).T
                ),  # [128, G]
                "wksum": np.ascontiguousarray(wksum[ks][:, None]),
                "wvsum": np.ascontiguousarray(wvsum[ks][:, None]),
                "bq": np.ascontiguousarray(bq_full[qs].reshape(G, 128).T),
                "bk": np.ascontiguousarray(bk_full[ks][:, None]),
                "bv": np.ascontiguousarray(bv_full[ks][:, None]),
            }
        )
    return in_maps, has_bias


def kernel(x, ln_gamma, ln_beta, Wq, Wk, Wv, Wo):
    from concourse.bass_utils import run_bass_kernel_spmd

    in_maps, has_bias = make_in_maps(x, ln_gamma, ln_beta, Wq, Wk, Wv, Wo)
    nc = get_program(has_bias)
    res = run_bass_kernel_spmd(nc, in_maps, core_ids=list(range(8)))
    outs = [m["outp"] for m in res.results]
    out = np.empty((B, S, H), np.float32)
    for b in range(B):
        out[b] = (outs[NKV * b] + outs[NKV * b + 1]) + (
            outs[NKV * b + 2] + outs[NKV * b + 3]
        )
    return out
